# revision 14
# baseline (speedup 1.0000x reference)
"""Bass/Trainium2 kernel for nn_DCDicl (DSBlock forward).

Algorithm: instead of the O(K^2 * R) unfold-Gram (baseline), compute the
all-pairs shift correlation corr[j,i,u,v] = sum_{h,w} x[j,h,w] *
xpad[i,h+u-4,w+v-4] (8x fewer FLOPs — the Gram is a Toeplitz gather of
corr), plus the U^T y rows for P folded into the same matmuls.

Device (8 cores = 4 samples x 2 w-halves, bf16 in / fp32 psum):
  out[m, (u,i,v)] = sum_{h, w in half} XY[m,h,w] * xpad[i, h+u, w+v]
with contraction over h (96 partitions) and PSUM accumulation over w.
Host: sum halves, gather Q via a sliding-window view, fp32 Cholesky solve.
"""

import sys
import time

import numpy as np

if "/opt/trn_rl_repo" not in sys.path:
    sys.path.append("/opt/trn_rl_repo")

N, C_IN, C_OUT, H, W, DS = 4, 64, 4, 96, 96, 5
K = C_IN * DS * DS          # 1600
NU = 2 * DS - 1             # 9 shifts per axis
M = C_IN + C_OUT            # 68 lhs rows (64 x-channels + 4 y-channels)
WH = W // 2                 # 48 w-columns per core (contraction half)
WV = WH + NU - 1            # 56 w-columns of padded image needed per core
HP = H + 2 * (DS - 1)       # 104 padded rows
NBLK = C_IN + M             # 132 56-wide column blocks (64 image + 68 xys)
COLS = NBLK * WV            # 7392 columns of the packed input
NCORES = 8

_CACHED = {}
_TIMING = True


def _mark(t, name):
    if _TIMING:
        now = time.perf_counter()
        print(f"[phase] {name}: {now - t[0]:.3f}s", file=sys.stderr)
        t[0] = now


def _build_nc():
    import concourse.bass as bass
    import concourse.mybir as mybir
    from concourse.tile import TileContext

    nc = bass.Bass()
    inp = nc.dram_tensor("inp", [HP, COLS], mybir.dt.bfloat16, kind="ExternalInput")
    out = nc.dram_tensor("o", [M, NU * C_IN * NU], mybir.dt.float32, kind="ExternalOutput")

    with TileContext(nc) as tc:
        with (
            tc.tile_pool(name="inp_p", bufs=1) as inp_p,
            tc.tile_pool(name="ps_p", bufs=8, space="PSUM") as ps_p,
            tc.tile_pool(name="st_p", bufs=1) as st_p,
        ):
            # One DMA materializes all 9 u-shifted replicas via an
            # overlapping sliding-window source AP: all_t[h, u, b, w] =
            # inp[h+u, b, w].  A single DMA completion sem keeps every
            # matmul at <=1 attached sync wait (the HW limit).
            all_t = inp_p.tile([H, NU, NBLK, WV], mybir.dt.bfloat16)
            src = inp[:, :]
            v = src.ap
            v.clear()
            v.extend([(COLS, H), (COLS, NU), (WV, NBLK), (1, WV)])
            nc.sync.dma_start(out=all_t[:, :, :, :], in_=src)

            stage = st_p.tile([M, NU * C_IN * NU], mybir.dt.float32)
            for u in range(NU):
                for ihalf in range(2):
                    ps = ps_p.tile([M, 32 * NU], mybir.dt.float32)
                    for wl in range(WH):
                        nc.tensor.matmul(
                            ps[:, :],
                            all_t[:, 0, C_IN:C_IN + M, wl],
                            all_t[:, u, ihalf * 32:(ihalf + 1) * 32, wl:wl + NU],
                            start=(wl == 0),
                            stop=(wl == WH - 1),
                        )
                    col = (u * 2 + ihalf) * 32 * NU
                    nc.vector.tensor_copy(stage[:, col:col + 32 * NU], ps[:, :])
            nc.sync.dma_start(out=out[:, :], in_=stage[:, :])

    _split_multiwait_drains(nc)
    return nc


def _split_multiwait_drains(nc):
    """Walrus rejects instructions carrying more than one attached sync wait.

    Tile's kernel-tail drain waits on every outstanding semaphore in one
    instruction; split it into a chain of single-wait drains.
    """
    import copy

    import concourse.mybir as mybir

    for fobj in nc.m.functions:
        for blk in fobj.blocks:
            insts = blk.instructions
            k = 0
            while k < len(insts):
                inst = insts[k]
                si = inst.sync_info
                if (
                    isinstance(inst, mybir.InstDrain)
                    and si is not None
                    and len(si.on_wait) > 1
                ):
                    waits = list(si.on_wait)
                    for j, w in enumerate(waits[:-1]):
                        d = copy.copy(inst)
                        d.name = f"{inst.name}_w{j}"
                        d.sync_info = mybir.SyncInfo(on_wait=[w], on_update=[])
                        nc.register_instruction(d)
                        insts.insert(k, d)
                        k += 1
                    inst.sync_info = mybir.SyncInfo(
                        on_wait=[waits[-1]], on_update=list(si.on_update)
                    )
                k += 1


def _build_runner():
    """Build the bass module once and return a cached jitted SPMD callable.

    Mirrors bass2jax.run_bass_via_pjrt's multi-core path, but the jitted
    shard_map is constructed a single time so later calls skip
    trace/lower/compile entirely.
    """
    import jax
    import concourse.mybir as mybir
    from concourse.bass2jax import (
        _bass_exec_p,
        install_neuronx_cc_hook,
        partition_id_tensor,
    )
    from jax.experimental.shard_map import shard_map
    from jax.sharding import Mesh, PartitionSpec

    nc = _build_nc()
    if not nc.is_finalized():
        nc.finalize()
    install_neuronx_cc_hook()
    assert nc.dbg_addr is None
    partition_name = (
        nc.partition_id_tensor.name if nc.partition_id_tensor is not None else None
    )

    in_names, out_names, out_avals, zero_shapes = [], [], [], []
    for alloc in nc.m.functions[0].allocations:
        if not isinstance(alloc, mybir.MemoryLocationSet):
            continue
        name = alloc.memorylocations[0].name
        if alloc.kind == "ExternalInput":
            if name != partition_name:
                in_names.append(name)
        elif alloc.kind == "ExternalOutput":
            shape = tuple(alloc.tensor_shape)
            dtype = mybir.dt.np(alloc.dtype)
            out_names.append(name)
            out_avals.append(jax.core.ShapedArray(shape, dtype))
            zero_shapes.append((shape, dtype))
    n_params = len(in_names)
    n_outs = len(out_avals)
    all_names = in_names + out_names
    if partition_name is not None:
        all_names = all_names + [partition_name]

    def _body(*args):
        operands = list(args)
        if partition_name is not None:
            operands.append(partition_id_tensor())
        outs = _bass_exec_p.bind(
            *operands,
            out_avals=tuple(out_avals),
            in_names=tuple(all_names),
            out_names=tuple(out_names),
            lowering_input_output_aliases=(),
            sim_require_finite=True,
            sim_require_nnan=True,
            nc=nc,
        )
        return tuple(outs)

    devices = jax.devices()[:NCORES]
    mesh = Mesh(np.asarray(devices), ("core",))
    donate = tuple(range(n_params, n_params + n_outs))
    sharded = jax.jit(
        shard_map(
            _body,
            mesh=mesh,
            in_specs=(PartitionSpec("core"),) * (n_params + n_outs),
            out_specs=(PartitionSpec("core"),) * n_outs,
            check_rep=False,
        ),
        donate_argnums=donate,
        keep_unused=True,
    )

    def run(in_maps):
        t = [time.perf_counter()]
        concat_in = [
            np.concatenate([np.asarray(m[name]) for m in in_maps], axis=0)
            for name in in_names
        ]
        concat_zeros = [
            np.zeros((NCORES * s[0], *s[1:]), dt) for s, dt in zero_shapes
        ]
        _mark(t, "  run.concat")
        out_arrs = sharded(*concat_in, *concat_zeros)
        _mark(t, "  run.dispatch")
        for a in out_arrs:
            a.block_until_ready()
        _mark(t, "  run.exec")
        res = [
            np.asarray(out_arrs[i]).reshape(NCORES, *out_avals[i].shape)
            for i in range(n_outs)
        ]
        _mark(t, "  run.fetch")
        return res

    return run


def _unfold(x1):
    """x1: [C_in, H, W] -> U [10000, 1600] (kept for test.py's oracle)."""
    from numpy.lib.stride_tricks import sliding_window_view

    xp2 = np.pad(x1, ((0, 0), (4, 4), (4, 4)))
    sw = sliding_window_view(xp2, (DS, DS), axis=(1, 2))
    return np.ascontiguousarray(
        sw.transpose(1, 2, 0, 3, 4).reshape(100 * 100, K), dtype=np.float32
    )


def _prep_in_maps(x, y):
    import ml_dtypes

    bf16 = ml_dtypes.bfloat16
    in_maps = []
    for s in range(N):
        xs = x[s, 0]
        ys = y[s, :, 0]
        xy = np.concatenate([xs, ys], axis=0)                   # [68, 96, 96]
        xyT = xy.transpose(1, 0, 2)                             # [96, 68, 96]
        xpad = np.zeros((C_IN, HP, HP), np.float32)
        xpad[:, DS - 1:DS - 1 + H, DS - 1:DS - 1 + W] = xs
        xpfT = xpad.transpose(1, 0, 2)                          # [104, 64, 104]
        for half in range(2):
            packed = np.zeros((HP, NBLK, WV), np.float32)
            packed[:, :C_IN, :] = xpfT[:, :, WH * half:WH * half + WV]
            packed[:H, C_IN:, :WH] = xyT[:, :, WH * half:WH * (half + 1)]
            in_maps.append({"inp": packed.reshape(HP, COLS).astype(bf16)})
    return in_maps


def kernel(x, d, y, alpha, reg):
    from numpy.lib.stride_tricks import sliding_window_view
    from scipy.linalg import cho_factor, cho_solve

    t = [time.perf_counter()]
    x = np.asarray(x, dtype=np.float32)
    d = np.asarray(d, dtype=np.float32)
    y = np.asarray(y, dtype=np.float32)
    alpha = np.asarray(alpha, dtype=np.float32)
    reg = np.asarray(reg, dtype=np.float32)

    if "run" not in _CACHED:
        _CACHED["run"] = _build_runner()
    run = _CACHED["run"]
    _mark(t, "build")

    in_maps = _prep_in_maps(x, y)
    _mark(t, "prep")

    res = run(in_maps)[0]                                        # [8, 68, 5184]
    _mark(t, "spmd_run")

    a = alpha.reshape(N) * H * W * float(reg[0]) / (DS * DS * C_IN)
    out = np.empty((N, C_OUT, C_IN, DS, DS), dtype=np.float32)
    for s in range(N):
        o = res[2 * s] + res[2 * s + 1]                          # [68, 5184]
        # columns are (u, ihalf, i_local, v) -> [m, i, u, v]
        oc = np.ascontiguousarray(
            o.reshape(M, NU, 2, 32, NU).transpose(0, 2, 3, 1, 4)
        ).reshape(M, C_IN, NU, NU)
        corr = oc[:C_IN]                                         # [j, i, u, v]
        p2 = oc[C_IN:]                                           # [co, i, u, v]

        # Q[(j,kh,kw),(i,ph,pw)] = corr[j, i, ph-kh+4, pw-kw+4]
        swv = sliding_window_view(corr, (DS, DS), axis=(2, 3))   # [j,i,a,b,ph,pw]
        Q4 = swv[:, :, ::-1, ::-1, :, :].transpose(0, 2, 3, 1, 4, 5)
        Q = np.ascontiguousarray(Q4).reshape(K, K)
        Q.flat[::K + 1] += a[s]

        P = np.ascontiguousarray(
            p2[:, :, DS - 3:DS + 2, DS - 3:DS + 2].transpose(1, 2, 3, 0)
        ).reshape(K, C_OUT)
        P += a[s] * d[s].transpose(1, 2, 3, 0).reshape(K, C_OUT)

        cf = cho_factor(Q, lower=False, check_finite=False)
        D = cho_solve(cf, P, check_finite=False)
        out[s] = D.reshape(C_IN, DS, DS, C_OUT).transpose(3, 0, 1, 2)
    _mark(t, "host_post")
    return out


# revision 19
# speedup vs baseline: 1.3166x; 1.3166x over previous
"""Bass/Trainium2 kernel for nn_DCDicl (DSBlock forward).

Algorithm: instead of the O(K^2 * R) unfold-Gram (baseline), compute the
all-pairs shift correlation corr[j,i,u,v] = sum_{h,w} x[j,h,w] *
xpad[i,h+u-4,w+v-4] (8x fewer FLOPs — the Gram is a Toeplitz gather of
corr), plus the U^T y rows for P folded into the same matmuls.

Device (8 cores = 4 samples x 2 w-halves, bf16 in / fp32 psum):
  out[m, (u,i,v)] = sum_{h, w in half} XY[m,h,w] * xpad[i, h+u, w+v]
with contraction over h (96 partitions) and PSUM accumulation over w.
Host: sum halves, gather Q via a sliding-window view, fp32 Cholesky solve.
"""

import sys
import time

import numpy as np

if "/opt/trn_rl_repo" not in sys.path:
    sys.path.append("/opt/trn_rl_repo")

N, C_IN, C_OUT, H, W, DS = 4, 64, 4, 96, 96, 5
K = C_IN * DS * DS          # 1600
NU = 2 * DS - 1             # 9 shifts per axis
M = C_IN + C_OUT            # 68 lhs rows (64 x-channels + 4 y-channels)
WH = W // 2                 # 48 w-columns per core (contraction half)
WV = WH + NU - 1            # 56 w-columns of padded image needed per core
HP = H + 2 * (DS - 1)       # 104 padded rows
NBLK = C_IN + M             # 132 56-wide column blocks (64 image + 68 xys)
COLS = NBLK * WV            # 7392 columns of the packed input
NUK = 7                     # computed u-shifts 0..6 (7,8 come from symmetry)
UF = 5                      # u-shifts with full [j,co] output
GW = 32 * NU                # 288 columns per accumulation group
NCORES = 8

_CACHED = {}
_TIMING = True


def _mark(t, name):
    if _TIMING:
        now = time.perf_counter()
        print(f"[phase] {name}: {now - t[0]:.3f}s", file=sys.stderr)
        t[0] = now


def _build_nc():
    import concourse.bass as bass
    import concourse.mybir as mybir
    from concourse.tile import TileContext

    nc = bass.Bass()
    inp = nc.dram_tensor("inp", [HP, COLS], mybir.dt.bfloat16, kind="ExternalInput")
    out1 = nc.dram_tensor("o1", [M, UF * 2 * GW], mybir.dt.bfloat16, kind="ExternalOutput")
    out2 = nc.dram_tensor(
        "o2", [C_OUT, (NUK - UF) * 2 * GW], mybir.dt.bfloat16, kind="ExternalOutput"
    )

    with TileContext(nc) as tc:
        with (
            tc.tile_pool(name="inp_p", bufs=1) as inp_p,
            tc.tile_pool(name="ps_p", bufs=8, space="PSUM") as ps_p,
            tc.tile_pool(name="st_p", bufs=1) as st_p,
        ):
            # One DMA materializes all 7 u-shifted replicas via an
            # overlapping sliding-window source AP: all_t[h, u, b, w] =
            # inp[h+u, b, w].  A single DMA completion sem keeps every
            # matmul at <=1 attached sync wait (the HW limit).
            all_t = inp_p.tile([H, NUK, NBLK, WV], mybir.dt.bfloat16)
            src = inp[:, :]
            v = src.ap
            v.clear()
            v.extend([(COLS, H), (COLS, NUK), (WV, NBLK), (1, WV)])
            nc.sync.dma_start(out=all_t[:, :, :, :], in_=src)

            stage1 = st_p.tile([M, UF * 2 * GW], mybir.dt.bfloat16)
            stage2 = st_p.tile([M, (NUK - UF) * 2 * GW], mybir.dt.bfloat16)
            for u in range(NUK):
                for ihalf in range(2):
                    ps = ps_p.tile([M, GW], mybir.dt.float32)
                    for wl in range(WH):
                        nc.tensor.matmul(
                            ps[:, :],
                            all_t[:, 0, C_IN:C_IN + M, wl],
                            all_t[:, u, ihalf * 32:(ihalf + 1) * 32, wl:wl + NU],
                            start=(wl == 0),
                            stop=(wl == WH - 1),
                        )
                    if u < UF:
                        col = (u * 2 + ihalf) * GW
                        nc.vector.tensor_copy(stage1[:, col:col + GW], ps[:, :])
                    else:
                        col = ((u - UF) * 2 + ihalf) * GW
                        nc.vector.tensor_copy(
                            stage2[C_IN:M, col:col + GW], ps[C_IN:M, :]
                        )
            nc.sync.dma_start(out=out1[:, :], in_=stage1[:, :])
            nc.sync.dma_start(out=out2[:, :], in_=stage2[C_IN:M, :])

    _split_multiwait_drains(nc)
    return nc


def _split_multiwait_drains(nc):
    """Walrus rejects instructions carrying more than one attached sync wait.

    Tile's kernel-tail drain waits on every outstanding semaphore in one
    instruction; split it into a chain of single-wait drains.
    """
    import copy

    import concourse.mybir as mybir

    for fobj in nc.m.functions:
        for blk in fobj.blocks:
            insts = blk.instructions
            k = 0
            while k < len(insts):
                inst = insts[k]
                si = inst.sync_info
                if (
                    isinstance(inst, mybir.InstDrain)
                    and si is not None
                    and len(si.on_wait) > 1
                ):
                    waits = list(si.on_wait)
                    for j, w in enumerate(waits[:-1]):
                        d = copy.copy(inst)
                        d.name = f"{inst.name}_w{j}"
                        d.sync_info = mybir.SyncInfo(on_wait=[w], on_update=[])
                        nc.register_instruction(d)
                        insts.insert(k, d)
                        k += 1
                    inst.sync_info = mybir.SyncInfo(
                        on_wait=[waits[-1]], on_update=list(si.on_update)
                    )
                k += 1


def _build_runner():
    """Build the bass module once and return a cached jitted SPMD callable.

    Mirrors bass2jax.run_bass_via_pjrt's multi-core path, but the jitted
    shard_map is constructed a single time so later calls skip
    trace/lower/compile entirely.
    """
    import jax
    import concourse.mybir as mybir
    from concourse.bass2jax import (
        _bass_exec_p,
        install_neuronx_cc_hook,
        partition_id_tensor,
    )
    from jax.experimental.shard_map import shard_map
    from jax.sharding import Mesh, PartitionSpec

    nc = _build_nc()
    if not nc.is_finalized():
        nc.finalize()
    install_neuronx_cc_hook()
    assert nc.dbg_addr is None
    partition_name = (
        nc.partition_id_tensor.name if nc.partition_id_tensor is not None else None
    )

    in_names, out_names, out_avals, zero_shapes = [], [], [], []
    for alloc in nc.m.functions[0].allocations:
        if not isinstance(alloc, mybir.MemoryLocationSet):
            continue
        name = alloc.memorylocations[0].name
        if alloc.kind == "ExternalInput":
            if name != partition_name:
                in_names.append(name)
        elif alloc.kind == "ExternalOutput":
            shape = tuple(alloc.tensor_shape)
            dtype = mybir.dt.np(alloc.dtype)
            out_names.append(name)
            out_avals.append(jax.core.ShapedArray(shape, dtype))
            zero_shapes.append((shape, dtype))
    n_params = len(in_names)
    n_outs = len(out_avals)
    all_names = in_names + out_names
    if partition_name is not None:
        all_names = all_names + [partition_name]

    def _body(*args):
        operands = list(args)
        if partition_name is not None:
            operands.append(partition_id_tensor())
        outs = _bass_exec_p.bind(
            *operands,
            out_avals=tuple(out_avals),
            in_names=tuple(all_names),
            out_names=tuple(out_names),
            lowering_input_output_aliases=(),
            sim_require_finite=True,
            sim_require_nnan=True,
            nc=nc,
        )
        return tuple(outs)

    devices = jax.devices()[:NCORES]
    mesh = Mesh(np.asarray(devices), ("core",))
    donate = tuple(range(n_params, n_params + n_outs))
    sharded = jax.jit(
        shard_map(
            _body,
            mesh=mesh,
            in_specs=(PartitionSpec("core"),) * (n_params + n_outs),
            out_specs=(PartitionSpec("core"),) * n_outs,
            check_rep=False,
        ),
        donate_argnums=donate,
        keep_unused=True,
    )

    # The donated output-seed buffers never leave the device: a jitted
    # sharded zeros-maker replaces an 11MB host->device upload per call.
    import jax.numpy as jnp
    from jax.sharding import NamedSharding

    zeros_sharding = tuple(
        NamedSharding(mesh, PartitionSpec("core")) for _ in zero_shapes
    )
    zeros_fn = jax.jit(
        lambda: tuple(
            jnp.zeros((NCORES * s[0], *s[1:]), dt) for s, dt in zero_shapes
        ),
        out_shardings=zeros_sharding,
    )

    def run(in_maps):
        t = [time.perf_counter()]
        concat_in = [
            np.concatenate([np.asarray(m[name]) for m in in_maps], axis=0)
            for name in in_names
        ]
        zeros = zeros_fn()
        _mark(t, "  run.concat")
        out_arrs = sharded(*concat_in, *zeros)
        _mark(t, "  run.dispatch")
        for a in out_arrs:
            a.block_until_ready()
        _mark(t, "  run.exec")
        res = [
            np.asarray(out_arrs[i]).reshape(NCORES, *out_avals[i].shape)
            for i in range(n_outs)
        ]
        _mark(t, "  run.fetch")
        return res

    return run


def _unfold(x1):
    """x1: [C_in, H, W] -> U [10000, 1600] (kept for test.py's oracle)."""
    from numpy.lib.stride_tricks import sliding_window_view

    xp2 = np.pad(x1, ((0, 0), (4, 4), (4, 4)))
    sw = sliding_window_view(xp2, (DS, DS), axis=(1, 2))
    return np.ascontiguousarray(
        sw.transpose(1, 2, 0, 3, 4).reshape(100 * 100, K), dtype=np.float32
    )


def _prep_in_maps(x, y):
    import ml_dtypes

    bf16 = ml_dtypes.bfloat16
    in_maps = []
    for s in range(N):
        xs = x[s, 0]
        ys = y[s, :, 0]
        xy = np.concatenate([xs, ys], axis=0)                   # [68, 96, 96]
        xyT = xy.transpose(1, 0, 2)                             # [96, 68, 96]
        xpad = np.zeros((C_IN, HP, HP), np.float32)
        xpad[:, DS - 1:DS - 1 + H, DS - 1:DS - 1 + W] = xs
        xpfT = xpad.transpose(1, 0, 2)                          # [104, 64, 104]
        for half in range(2):
            packed = np.zeros((HP, NBLK, WV), np.float32)
            packed[:, :C_IN, :] = xpfT[:, :, WH * half:WH * half + WV]
            packed[:H, C_IN:, :WH] = xyT[:, :, WH * half:WH * (half + 1)]
            in_maps.append({"inp": packed.reshape(HP, COLS).astype(bf16)})
    return in_maps


def kernel(x, d, y, alpha, reg):
    from numpy.lib.stride_tricks import sliding_window_view
    from scipy.linalg import cho_factor, cho_solve

    t = [time.perf_counter()]
    x = np.asarray(x, dtype=np.float32)
    d = np.asarray(d, dtype=np.float32)
    y = np.asarray(y, dtype=np.float32)
    alpha = np.asarray(alpha, dtype=np.float32)
    reg = np.asarray(reg, dtype=np.float32)

    if "run" not in _CACHED:
        _CACHED["run"] = _build_runner()
    run = _CACHED["run"]
    _mark(t, "build")

    in_maps = _prep_in_maps(x, y)
    _mark(t, "prep")

    res1, res2 = run(in_maps)            # [8, 68, 2880] bf16, [8, 4, 1152] bf16
    _mark(t, "spmd_run")

    a = alpha.reshape(N) * H * W * float(reg[0]) / (DS * DS * C_IN)
    out = np.empty((N, C_OUT, C_IN, DS, DS), dtype=np.float32)

    def _solve(s):
        o1 = np.asarray(res1[2 * s], np.float32) + np.asarray(res1[2 * s + 1], np.float32)
        o2 = np.asarray(res2[2 * s], np.float32) + np.asarray(res2[2 * s + 1], np.float32)
        # o1 columns are (u<5, ihalf, i_local, v) -> [m, i, u, v]
        oc1 = np.ascontiguousarray(
            o1.reshape(M, UF, 2, 32, NU).transpose(0, 2, 3, 1, 4)
        ).reshape(M, C_IN, UF, NU)
        # corr[j,i,u,v]; u>=5 from symmetry corr[j,i,u,v] = corr[i,j,8-u,8-v]
        corr = np.empty((C_IN, C_IN, NU, NU), np.float32)
        cl = oc1[:C_IN]
        corr[:, :, :UF, :] = cl
        corr[:, :, UF:, :] = np.flip(
            cl.transpose(1, 0, 2, 3)[:, :, :NU - UF, :], axis=(2, 3)
        )

        # Q[(j,kh,kw),(i,ph,pw)] = corr[j, i, ph-kh+4, pw-kw+4]
        swv = sliding_window_view(corr, (DS, DS), axis=(2, 3))   # [j,i,a,b,ph,pw]
        Q4 = swv[:, :, ::-1, ::-1, :, :].transpose(0, 2, 3, 1, 4, 5)
        Q = np.ascontiguousarray(Q4).reshape(K, K)
        Q.flat[::K + 1] += a[s]

        # P2[co, i, u, v] for u in 2..6: u<=4 from o1 rows 64:, u in {5,6} from o2
        oc2 = np.ascontiguousarray(
            o2.reshape(C_OUT, NUK - UF, 2, 32, NU).transpose(0, 2, 3, 1, 4)
        ).reshape(C_OUT, C_IN, NUK - UF, NU)
        p2u = np.concatenate([oc1[C_IN:][:, :, 2:UF, :], oc2], axis=2)
        P = np.ascontiguousarray(
            p2u[:, :, :, DS - 3:DS + 2].transpose(1, 2, 3, 0)
        ).reshape(K, C_OUT)
        P += a[s] * d[s].transpose(1, 2, 3, 0).reshape(K, C_OUT)

        cf = cho_factor(Q, lower=False, check_finite=False)
        D = cho_solve(cf, P, check_finite=False)
        out[s] = D.reshape(C_IN, DS, DS, C_OUT).transpose(3, 0, 1, 2)

    from concurrent.futures import ThreadPoolExecutor

    with ThreadPoolExecutor(max_workers=N) as ex:
        list(ex.map(_solve, range(N)))
    _mark(t, "host_post")
    return out


# revision 26
# speedup vs baseline: 1.5874x; 1.2057x over previous
"""Bass/Trainium2 kernel for nn_DCDicl (DSBlock forward).

Algorithm: instead of the O(K^2 * R) unfold-Gram (baseline), compute the
all-pairs shift correlation corr[j,i,u,v] = sum_{h,w} x[j,h,w] *
xpad[i,h+u-4,w+v-4] (8x fewer FLOPs — the Gram is a Toeplitz gather of
corr), plus the U^T y rows for P folded into the same matmuls.

Device (8 cores = 4 samples x 2 w-halves, bf16 in / fp32 psum):
  out[m, (u,i,v)] = sum_{h, w in half} XY[m,h,w] * xpad[i, h+u, w+v]
with contraction over h (96 partitions) and PSUM accumulation over w.
Host: sum halves, gather Q via a sliding-window view, fp32 Cholesky solve.
"""

import sys
import time

import numpy as np

if "/opt/trn_rl_repo" not in sys.path:
    sys.path.append("/opt/trn_rl_repo")

N, C_IN, C_OUT, H, W, DS = 4, 64, 4, 96, 96, 5
K = C_IN * DS * DS          # 1600
NU = 2 * DS - 1             # 9 shifts per axis
M = C_IN + C_OUT            # 68 lhs rows (64 x-channels + 4 y-channels)
WH = W // 2                 # 48 w-columns per core (contraction half)
WV = WH + NU - 1            # 56 w-columns of padded image needed per core
HP = H + 2 * (DS - 1)       # 104 padded rows
NBLK = C_IN + C_OUT         # 68 56-wide column blocks (64 padded-x + 4 y)
COLS = NBLK * WV            # 3808 columns of the packed input
NUK = 7                     # computed u-shifts 0..6 (7,8 come from symmetry)
UF = 5                      # u-shifts computed for the x-x correlation
GW = 32 * NU                # 288 columns per x-corr accumulation group
GY = C_IN * DS              # 320 columns per y-corr accumulation group
NCORES = 8

_CACHED = {}
_TIMING = True


def _mark(t, name):
    if _TIMING:
        now = time.perf_counter()
        print(f"[phase] {name}: {now - t[0]:.3f}s", file=sys.stderr)
        t[0] = now


def _build_nc():
    import concourse.bass as bass
    import concourse.mybir as mybir
    from concourse.tile import TileContext

    nc = bass.Bass()
    inp = nc.dram_tensor("inp", [HP, COLS], mybir.dt.bfloat16, kind="ExternalInput")
    out1 = nc.dram_tensor("o1", [C_IN, UF * 2 * GW], mybir.dt.bfloat16, kind="ExternalOutput")
    out2 = nc.dram_tensor("o2", [C_OUT, DS * GY], mybir.dt.bfloat16, kind="ExternalOutput")

    with TileContext(nc) as tc:
        with (
            tc.tile_pool(name="inp_p", bufs=1) as inp_p,
            tc.tile_pool(name="ps_p", bufs=6, space="PSUM") as ps_p,
            tc.tile_pool(name="py_p", bufs=2, space="PSUM") as py_p,
            tc.tile_pool(name="st_p", bufs=1) as st_p,
        ):
            # One DMA materializes all 7 u-shifted replicas via an
            # overlapping sliding-window source AP: all_t[h, u, b, w] =
            # inp[h+u, b, w].  A single DMA completion sem keeps every
            # matmul at <=1 attached sync wait (the HW limit).  The
            # unpadded x itself (the matmul lhsT) is the interior of the
            # u=4 replica, so x is shipped only once.
            all_t = inp_p.tile([H, NUK, NBLK, WV], mybir.dt.bfloat16)
            src = inp[:, :]
            v = src.ap
            v.clear()
            v.extend([(COLS, H), (COLS, NUK), (WV, NBLK), (1, WV)])
            nc.sync.dma_start(out=all_t[:, :, :, :], in_=src)

            stage1 = st_p.tile([C_IN, UF * 2 * GW], mybir.dt.bfloat16)
            stage2 = st_p.tile([C_OUT, DS * GY], mybir.dt.bfloat16)
            # x-x correlation: corr[j, i, u, v], u in 0..4 (rest by symmetry)
            for u in range(UF):
                for ihalf in range(2):
                    ps = ps_p.tile([C_IN, GW], mybir.dt.float32)
                    for wl in range(WH):
                        nc.tensor.matmul(
                            ps[:, :],
                            all_t[:, 4, 0:C_IN, wl + 4],
                            all_t[:, u, ihalf * 32:(ihalf + 1) * 32, wl:wl + NU],
                            start=(wl == 0),
                            stop=(wl == WH - 1),
                        )
                    col = (u * 2 + ihalf) * GW
                    nc.vector.tensor_copy(stage1[:, col:col + GW], ps[:, :])
            # y-x correlation: p2[co, i, u, v], u in 2..6, v in 2..6
            for ui in range(DS):
                psy = py_p.tile([C_OUT, GY], mybir.dt.float32)
                for wl in range(WH):
                    nc.tensor.matmul(
                        psy[:, :],
                        all_t[:, 0, C_IN:NBLK, wl],
                        all_t[:, ui + 2, 0:C_IN, wl + 2:wl + 2 + DS],
                        start=(wl == 0),
                        stop=(wl == WH - 1),
                    )
                nc.vector.tensor_copy(stage2[:, ui * GY:(ui + 1) * GY], psy[:, :])
            nc.sync.dma_start(out=out1[:, :], in_=stage1[:, :])
            nc.sync.dma_start(out=out2[:, :], in_=stage2[:, :])

    _split_multiwait_drains(nc)
    return nc


def _split_multiwait_drains(nc):
    """Walrus rejects instructions carrying more than one attached sync wait.

    Tile's kernel-tail drain waits on every outstanding semaphore in one
    instruction; split it into a chain of single-wait drains.
    """
    import copy

    import concourse.mybir as mybir

    for fobj in nc.m.functions:
        for blk in fobj.blocks:
            insts = blk.instructions
            k = 0
            while k < len(insts):
                inst = insts[k]
                si = inst.sync_info
                if (
                    isinstance(inst, mybir.InstDrain)
                    and si is not None
                    and len(si.on_wait) > 1
                ):
                    waits = list(si.on_wait)
                    for j, w in enumerate(waits[:-1]):
                        d = copy.copy(inst)
                        d.name = f"{inst.name}_w{j}"
                        d.sync_info = mybir.SyncInfo(on_wait=[w], on_update=[])
                        nc.register_instruction(d)
                        insts.insert(k, d)
                        k += 1
                    inst.sync_info = mybir.SyncInfo(
                        on_wait=[waits[-1]], on_update=list(si.on_update)
                    )
                k += 1


def _build_runner():
    """Build the bass module once and return a cached jitted SPMD callable.

    Mirrors bass2jax.run_bass_via_pjrt's multi-core path, but the jitted
    shard_map is constructed a single time so later calls skip
    trace/lower/compile entirely.
    """
    import jax
    import concourse.mybir as mybir
    from concourse.bass2jax import (
        _bass_exec_p,
        install_neuronx_cc_hook,
        partition_id_tensor,
    )
    from jax.experimental.shard_map import shard_map
    from jax.sharding import Mesh, PartitionSpec

    nc = _build_nc()
    if not nc.is_finalized():
        nc.finalize()
    install_neuronx_cc_hook()
    assert nc.dbg_addr is None
    partition_name = (
        nc.partition_id_tensor.name if nc.partition_id_tensor is not None else None
    )

    in_names, out_names, out_avals, zero_shapes = [], [], [], []
    for alloc in nc.m.functions[0].allocations:
        if not isinstance(alloc, mybir.MemoryLocationSet):
            continue
        name = alloc.memorylocations[0].name
        if alloc.kind == "ExternalInput":
            if name != partition_name:
                in_names.append(name)
        elif alloc.kind == "ExternalOutput":
            shape = tuple(alloc.tensor_shape)
            dtype = mybir.dt.np(alloc.dtype)
            out_names.append(name)
            out_avals.append(jax.core.ShapedArray(shape, dtype))
            zero_shapes.append((shape, dtype))
    n_params = len(in_names)
    n_outs = len(out_avals)
    all_names = in_names + out_names
    if partition_name is not None:
        all_names = all_names + [partition_name]

    def _body(*args):
        operands = list(args)
        if partition_name is not None:
            operands.append(partition_id_tensor())
        outs = _bass_exec_p.bind(
            *operands,
            out_avals=tuple(out_avals),
            in_names=tuple(all_names),
            out_names=tuple(out_names),
            lowering_input_output_aliases=(),
            sim_require_finite=True,
            sim_require_nnan=True,
            nc=nc,
        )
        return tuple(outs)

    devices = jax.devices()[:NCORES]
    mesh = Mesh(np.asarray(devices), ("core",))
    donate = tuple(range(n_params, n_params + n_outs))
    sharded = jax.jit(
        shard_map(
            _body,
            mesh=mesh,
            in_specs=(PartitionSpec("core"),) * (n_params + n_outs),
            out_specs=(PartitionSpec("core"),) * n_outs,
            check_rep=False,
        ),
        donate_argnums=donate,
        keep_unused=True,
    )

    # The donated output-seed buffers never leave the device: a jitted
    # sharded zeros-maker replaces an 11MB host->device upload per call.
    import jax.numpy as jnp
    from jax.sharding import NamedSharding

    zeros_sharding = tuple(
        NamedSharding(mesh, PartitionSpec("core")) for _ in zero_shapes
    )
    zeros_fn = jax.jit(
        lambda: tuple(
            jnp.zeros((NCORES * s[0], *s[1:]), dt) for s, dt in zero_shapes
        ),
        out_shardings=zeros_sharding,
    )

    def run(in_maps):
        t = [time.perf_counter()]
        concat_in = [
            np.concatenate([np.asarray(m[name]) for m in in_maps], axis=0)
            for name in in_names
        ]
        zeros = zeros_fn()
        _mark(t, "  run.concat")
        out_arrs = sharded(*concat_in, *zeros)
        _mark(t, "  run.dispatch")
        for a in out_arrs:
            a.block_until_ready()
        _mark(t, "  run.exec")
        jobs = []
        for i, a in enumerate(out_arrs):
            rows = out_avals[i].shape[0]
            for sh in a.addressable_shards:
                c = sh.index[0].start // rows if sh.index[0].start else 0
                jobs.append((i, c, sh.data))

        def _fetch(job):
            i, c, data = job
            return i, c, np.asarray(data)

        res = [
            np.empty((NCORES, *out_avals[i].shape), out_avals[i].dtype)
            for i in range(n_outs)
        ]
        from concurrent.futures import ThreadPoolExecutor

        with ThreadPoolExecutor(max_workers=16) as ex:
            for i, c, arr in ex.map(_fetch, jobs):
                res[i][c] = arr.reshape(out_avals[i].shape)
        _mark(t, "  run.fetch")
        return res

    return run


def _unfold(x1):
    """x1: [C_in, H, W] -> U [10000, 1600] (kept for test.py's oracle)."""
    from numpy.lib.stride_tricks import sliding_window_view

    xp2 = np.pad(x1, ((0, 0), (4, 4), (4, 4)))
    sw = sliding_window_view(xp2, (DS, DS), axis=(1, 2))
    return np.ascontiguousarray(
        sw.transpose(1, 2, 0, 3, 4).reshape(100 * 100, K), dtype=np.float32
    )


def _prep_in_maps(x, y):
    import ml_dtypes

    bf16 = ml_dtypes.bfloat16
    in_maps = []
    for s in range(N):
        xs = x[s, 0]
        ys = y[s, :, 0]
        yT = ys.transpose(1, 0, 2)                              # [96, 4, 96]
        xpad = np.zeros((C_IN, HP, HP), np.float32)
        xpad[:, DS - 1:DS - 1 + H, DS - 1:DS - 1 + W] = xs
        xpfT = xpad.transpose(1, 0, 2)                          # [104, 64, 104]
        for half in range(2):
            packed = np.zeros((HP, NBLK, WV), np.float32)
            packed[:, :C_IN, :] = xpfT[:, :, WH * half:WH * half + WV]
            packed[:H, C_IN:, :WH] = yT[:, :, WH * half:WH * (half + 1)]
            in_maps.append({"inp": packed.reshape(HP, COLS).astype(bf16)})
    return in_maps


def kernel(x, d, y, alpha, reg):
    from numpy.lib.stride_tricks import sliding_window_view
    from scipy.linalg import cho_factor, cho_solve

    t = [time.perf_counter()]
    x = np.asarray(x, dtype=np.float32)
    d = np.asarray(d, dtype=np.float32)
    y = np.asarray(y, dtype=np.float32)
    alpha = np.asarray(alpha, dtype=np.float32)
    reg = np.asarray(reg, dtype=np.float32)

    if "run" not in _CACHED:
        _CACHED["run"] = _build_runner()
    run = _CACHED["run"]
    _mark(t, "build")

    in_maps = _prep_in_maps(x, y)
    _mark(t, "prep")

    res1, res2 = run(in_maps)            # [8, 68, 2880] bf16, [8, 4, 1152] bf16
    _mark(t, "spmd_run")

    a = alpha.reshape(N) * H * W * float(reg[0]) / (DS * DS * C_IN)
    out = np.empty((N, C_OUT, C_IN, DS, DS), dtype=np.float32)

    def _solve(s):
        o1 = np.asarray(res1[2 * s], np.float32) + np.asarray(res1[2 * s + 1], np.float32)
        o2 = np.asarray(res2[2 * s], np.float32) + np.asarray(res2[2 * s + 1], np.float32)
        # o1 columns are (u<5, ihalf, i_local, v) -> [j, i, u, v]
        cl = np.ascontiguousarray(
            o1.reshape(C_IN, UF, 2, 32, NU).transpose(0, 2, 3, 1, 4)
        ).reshape(C_IN, C_IN, UF, NU)
        # corr[j,i,u,v]; u>=5 from symmetry corr[j,i,u,v] = corr[i,j,8-u,8-v]
        corr = np.empty((C_IN, C_IN, NU, NU), np.float32)
        corr[:, :, :UF, :] = cl
        corr[:, :, UF:, :] = np.flip(
            cl.transpose(1, 0, 2, 3)[:, :, :NU - UF, :], axis=(2, 3)
        )

        # Q[(j,kh,kw),(i,ph,pw)] = corr[j, i, ph-kh+4, pw-kw+4]
        swv = sliding_window_view(corr, (DS, DS), axis=(2, 3))   # [j,i,a,b,ph,pw]
        Q4 = swv[:, :, ::-1, ::-1, :, :].transpose(0, 2, 3, 1, 4, 5)
        Q = np.ascontiguousarray(Q4).reshape(K, K)
        Q.flat[::K + 1] += a[s]

        # o2 columns are (u-2, i, v-2) for u,v in 2..6 -> P[(i,ph,pw), co]
        p2u = o2.reshape(C_OUT, DS, C_IN, DS)
        P = np.ascontiguousarray(p2u.transpose(2, 1, 3, 0)).reshape(K, C_OUT)
        P += a[s] * d[s].transpose(1, 2, 3, 0).reshape(K, C_OUT)

        cf = cho_factor(Q, lower=False, check_finite=False)
        D = cho_solve(cf, P, check_finite=False)
        out[s] = D.reshape(C_IN, DS, DS, C_OUT).transpose(3, 0, 1, 2)

    from concurrent.futures import ThreadPoolExecutor

    with ThreadPoolExecutor(max_workers=N) as ex:
        list(ex.map(_solve, range(N)))
    _mark(t, "host_post")
    return out
